# revision 10
# baseline (speedup 1.0000x reference)
"""Trainium2 Bass kernel for nn_BatchDelayProcessor.

Computes, per batch row (B=64, T=441000, D=22050 delay, 20 blocks):
    delayed[t] = 0                          , t < D
    delayed[t] = x[t-D] + 0.3*delayed[t-D]  , t >= D
    out[t]     = 0.5*x[t] + 0.5*delayed[t]

Unrolling the block recurrence, out_p = sum_j W[p,j] * x_j with the banded
lower-triangular W[p,p] = 0.5, W[p,j] = 0.5*0.3^(p-1-j) (j<p) -- i.e. a
20x20 matmul over the block axis, identical for every row.  Layout:
partition = (row-in-group, block) (4 rows/group -> 80 partitions), free =
sample offset; the PE does the whole recurrence as OUT = W @ X with a
block-diagonal stationary, bf16 in / f32 PSUM out.

v6 (from v1-v5 trace analysis):
  - x is cast f32->bf16 on the HOST (identical numerics to v1's in-DMA
    cast), and all x/y DRAM params + SBUF x/out tiles are DECLARED as
    uint32 (bf16 pairs).  Trace fits across v1-v5 show SWDGE moves ~1
    ELEMENT per partition per cycle regardless of element size (f32
    loads ran 406 GB/s, bf16 loads only 220 with identical element
    counts; an AP-level bitcast changed nothing, so the element size
    must come from the tensor/param declaration).  u32 elements halve
    the element count -> loads target ~16us instead of 32.
  - Engine-side APs bitcast back to bf16 (PE rhs, PSUM->SBUF copies).
  - Groups sequential; progressive slabs g0 [490(SP), 3920, 8820, 8820]
    / g1 [4410, 8820, 8820] keep the PE (36.7us busy at its 1.2 GHz
    pace, the critical path) fed from ~12us.  g0's 490-col slab rides
    the otherwise-idle SP HWDGE ring (fine for 78 KB) during the ~9.5us
    GpSimd boot window.
  - Stores: g0 [27,18] chunks; g1 [18,9,6,6,6] so the PE-gated final
    pieces drain concurrently with the last matmuls.

Engine split:
  GpSimd: 6 slab loads then 7 store pieces via SWDGE queue 0
  PE:     90 bf16 matmuls (80-partition block-diag W)
  DVE:    PSUM->SBUF bf16 copies, even matmul indices
  ACT:    PSUM->SBUF bf16 copies, odd matmul indices
  SP:     W load + g0's first 490-col slab (HWDGE, lands by ~7us)
"""

from contextlib import ExitStack

import numpy as np

import concourse.bass as bass
import concourse.mybir as mybir
from concourse.bass_utils import run_bass_kernel_spmd

B, T = 64, 441000
D, NBLK = 22050, 20
NCORES = 8
ROWS = B // NCORES          # 8 rows per core
GROUPS = 2                  # row groups per core
GR = ROWS // GROUPS         # 4 rows per group
P = GR * NBLK               # 80 partitions: (row-in-group, block)
MMCOL = 490                 # columns per matmul (<=512 psum bank cap)
NBANK = 8                   # PSUM banks in round-robin
TU = T // 2                 # u32 units per row
DU = D // 2                 # u32 units per block
MMU = MMCOL // 2            # u32 units per matmul chunk

# Per-group slab widths in bf16 cols (all even).  g0's leading 490 goes
# via SP/HWDGE so the PE starts early.
SLAB_W = [
    [490, 3920, 8820, 8820],   # g0 (slab 0 on the SP ring)
    [4410, 8820, 8820],        # g1
]
SLABS = []
for _ws in SLAB_W:
    _c, _sl = 0, []
    for _w in _ws:
        _sl.append((_c, _c + _w))
        _c += _w
    assert _c == D
    SLABS.append(_sl)
CHUNKS = [[(c1 - c0) // MMCOL for c0, c1 in sl] for sl in SLABS]
GROUP_MM = sum(CHUNKS[0])   # 45
assert sum(CHUNKS[1]) == GROUP_MM
NMM = GROUPS * GROUP_MM     # 90

# Store pieces per group, (start, end) in 490-col chunk units.  g1's
# fine taper overlaps the PE's tail.
ST_CHUNKS = [
    [(0, 27), (27, 45)],
    [(0, 18), (18, 27), (27, 33), (33, 39), (39, 45)],
]

F32 = mybir.dt.float32
BF16 = mybir.dt.bfloat16
U32 = mybir.dt.uint32


def _weights() -> np.ndarray:
    """lhsT for nc.tensor.matmul: out = lhsT.T @ rhs.

    lhsT[(r,j), (r',p)] = W[p, j] if r == r' else 0, with
    W[p, j] = 0.5*(p==j) + 0.5*0.3^(p-1-j)*(j<p).
    """
    W = np.zeros((NBLK, NBLK), np.float64)
    for p in range(NBLK):
        W[p, p] = 0.5
        for j in range(p):
            W[p, j] = 0.5 * 0.3 ** (p - 1 - j)
    import ml_dtypes

    return np.kron(np.eye(GR), W.T).astype(ml_dtypes.bfloat16)


def build_nc() -> bass.Bass:
    nc = bass.Bass(trn_type="TRN2")
    x = nc.declare_dram_parameter("x", [ROWS, TU], U32, isOutput=False)
    w = nc.declare_dram_parameter("w", [P, P], BF16, isOutput=False)
    y = nc.declare_dram_parameter("y", [ROWS, TU], U32, isOutput=True)
    xv = x.rearrange("r (j c) -> r j c", j=NBLK)   # (8, 20, 11025) u32
    yv = y.rearrange("r (j c) -> r j c", j=NBLK)

    with ExitStack() as ctx:
        block = ctx.enter_context(nc.Block())
        wbuf = ctx.enter_context(nc.sbuf_tensor("wbuf", [P, P], BF16))
        xbuf = [
            ctx.enter_context(nc.sbuf_tensor(f"xbuf{g}", [P, DU], U32))
            for g in range(GROUPS)
        ]
        obuf = [
            ctx.enter_context(nc.sbuf_tensor(f"obuf{g}", [P, DU], U32))
            for g in range(GROUPS)
        ]
        psum = [
            ctx.enter_context(nc.psum_tensor(f"ps{b}", [P, MMCOL], F32))
            for b in range(NBANK)
        ]
        s_w = ctx.enter_context(nc.semaphore("s_w"))
        s_x = [
            [
                ctx.enter_context(nc.semaphore(f"s_x{g}_{s}"))
                for s in range(len(SLABS[g]))
            ]
            for g in range(GROUPS)
        ]
        s_mm = ctx.enter_context(nc.semaphore("s_mm"))
        s_cpd = ctx.enter_context(nc.semaphore("s_cpd"))
        s_cpa = ctx.enter_context(nc.semaphore("s_cpa"))
        s_st = ctx.enter_context(nc.semaphore("s_st"))

        # copies done counts after copy idx: (# s_cpd incs, # s_cpa incs)
        def copies_done(last_idx):
            return (last_idx + 2) // 2, (last_idx + 1) // 2

        def load(eng, g, s):
            c0, c1 = SLABS[g][s]
            eng.dma_start(
                out=xbuf[g][:, c0 // 2 : c1 // 2],
                in_=xv[g * GR : (g + 1) * GR, :, c0 // 2 : c1 // 2],
            ).then_inc(s_x[g][s], 16)

        @block.sync
        def _(sp):
            # Both land during the GpSimd boot window (~9.5us): W (12.8
            # KB) then g0's 490-col slab (78 KB) -- small enough for the
            # slow (~25 GB/s) HWDGE path, and they unblock the PE early.
            sp.dma_start(out=wbuf[:, :], in_=w[:, :]).then_inc(s_w, 16)
            load(sp, 0, 0)

        @block.gpsimd
        def _(gp):
            # All q0 loads up front, zero waits (slab (0,0) rides SP).
            for g in range(GROUPS):
                for s in range(len(SLABS[g])):
                    if (g, s) != (0, 0):
                        load(gp, g, s)
            # Store pieces: chunk-aligned column ranges of each group's
            # obuf; DRAM side is 80 runs inside the contiguous 3.53 MB
            # range y[4g:4g+4, :].
            for g in range(GROUPS):
                for i0, i1 in ST_CHUNKS[g]:
                    nd, na = copies_done(g * GROUP_MM + i1 - 1)
                    gp.wait_ge(s_cpd, nd)
                    gp.wait_ge(s_cpa, na)
                    gp.dma_start(
                        out=yv[g * GR : (g + 1) * GR, :, i0 * MMU : i1 * MMU],
                        in_=obuf[g][:, i0 * MMU : i1 * MMU],
                    ).then_inc(s_st, 16)

        @block.tensor
        def _(te):
            te.wait_ge(s_w, 16)
            idx = 0
            for g in range(GROUPS):
                for s in range(len(SLABS[g])):
                    for i in range(CHUNKS[g][s]):
                        if i == 0:
                            te.wait_ge(s_x[g][s], 16)
                        if idx >= NBANK:
                            # PSUM bank WAR: copy idx-NBANK retired
                            old = idx - NBANK
                            if old % 2 == 0:
                                te.wait_ge(s_cpd, old // 2 + 1)
                            else:
                                te.wait_ge(s_cpa, old // 2 + 1)
                        u0 = (SLABS[g][s][0] + i * MMCOL) // 2
                        nc.tensor.matmul(
                            out=psum[idx % NBANK][:, :],
                            lhsT=wbuf[:, :],
                            rhs=xbuf[g][:, u0 : u0 + MMU].bitcast(BF16),
                            start=True,
                            stop=True,
                        ).then_inc(s_mm, 1)
                        idx += 1

        def _copy_prog(eng, vec, parity, sem):
            idx = 0
            for g in range(GROUPS):
                for s in range(len(SLABS[g])):
                    for i in range(CHUNKS[g][s]):
                        if idx % 2 == parity:
                            eng.wait_ge(s_mm, idx + 1)
                            u0 = (SLABS[g][s][0] + i * MMCOL) // 2
                            vec(
                                obuf[g][:, u0 : u0 + MMU].bitcast(BF16),
                                psum[idx % NBANK][:, :],
                            ).then_inc(sem, 1)
                        idx += 1

        @block.vector
        def _(ve):
            _copy_prog(ve, nc.vector.tensor_copy, 0, s_cpd)

        @block.scalar
        def _(sc):
            _copy_prog(sc, nc.scalar.copy, 1, s_cpa)

    return nc


_NC_CACHE = None


def _get_nc() -> bass.Bass:
    global _NC_CACHE
    if _NC_CACHE is None:
        _NC_CACHE = build_nc()
    return _NC_CACHE


_W = _weights()


def _shard(x: np.ndarray) -> list[dict[str, np.ndarray]]:
    import ml_dtypes

    x = np.asarray(x, dtype=np.float32)
    assert x.shape == (B, T), x.shape
    # Host-side f32 -> bf16 cast (same numerics as v1's in-DMA cast),
    # then view pairs of bf16 as uint32 for element-rate-limited DMA.
    xb = np.ascontiguousarray(x.astype(ml_dtypes.bfloat16)).view(np.uint32)
    return [
        {
            "x": np.ascontiguousarray(xb[i * ROWS : (i + 1) * ROWS]),
            "w": _W,
        }
        for i in range(NCORES)
    ]


def _unview(ys) -> np.ndarray:
    import ml_dtypes

    out = np.concatenate([np.asarray(r["y"]) for r in ys], axis=0)
    return out.view(ml_dtypes.bfloat16).astype(np.float32)


def kernel(x: np.ndarray) -> np.ndarray:
    nc = _get_nc()
    res = run_bass_kernel_spmd(nc, _shard(x), core_ids=list(range(NCORES)))
    return _unview(res.results)


def kernel_profiled(x: np.ndarray):
    """Like kernel() but with NTFF tracing; returns (out, BassKernelResults)."""
    nc = _get_nc()
    res = run_bass_kernel_spmd(
        nc, _shard(x), core_ids=list(range(NCORES)), trace=True
    )
    return _unview(res.results), res
